# revision 22
# baseline (speedup 1.0000x reference)
"""Trainium2 Bass kernel for nn_EqStftPBC (STFT perturbation-based compensation).

Per (batch b, mode m):
  X = STFT(x); C_n2 = X*conj(roll(X,n2)) + prev-frame; U_n2 = circulant(w[:,n2]);
  V_n2 = U_n2 * roll(X,n2); delta = sum_n2 V_n2; out = x + ISTFT(delta)*P (+bias)

8 cores = (b x m x n2-half), uniform SPMD program; per-core variation only in
input data (permutation stack S, circulant stack M).  Device layout: [freq(80)
partitions, time free].  STFT fp32, rest bf16 (PSUM fp32).
"""

import numpy as np
from ml_dtypes import bfloat16

import concourse.bass as bass
import concourse.bacc as bacc
import concourse.mybir as mybir
import concourse.tile as tile

F = 80
T = 51
TP = 52          # padded slot stride
HOP = 40
L = 2080
NJ = 20
NCH = 4
CHJ = NJ // NCH
PBK = 5          # stage-1/R psum outputs per bank
GJ = 5           # j per merged G-matmul (N = GJ*102 <= 512)
FP32 = mybir.dt.float32
BF16 = mybir.dt.bfloat16

N2_LISTS = [list(range(19, -1, -1)), list(range(-1, -21, -1))]


def _dft_consts():
    j = np.arange(F)
    W = np.exp(-2j * np.pi * np.outer(j, j) / F)
    G = np.exp(+2j * np.pi * np.outer(j, j) / F) / F
    return W, G


def build_program(debug=False):
    nc = bacc.Bacc("TRN2", target_bir_lowering=False, debug=debug)

    # xf = [fiN | fr | fi] frames, pre-framed on host (pure reshape)
    xf = nc.dram_tensor("xf", [F, 3 * T], FP32, kind="ExternalInput")
    fr_c = nc.dram_tensor("fr_c", [F, 2 * F], FP32, kind="ExternalInput")
    gr_c = nc.dram_tensor("gr_c", [F, 2 * F], BF16, kind="ExternalInput")
    smat = nc.dram_tensor("smat", [F, NJ * F], BF16, kind="ExternalInput")
    mst = nc.dram_tensor("mst", [F, NJ * 2 * F], BF16, kind="ExternalInput")
    svec = nc.dram_tensor("svec", [HOP, 52], FP32, kind="ExternalInput")
    yv = nc.dram_tensor("yv", [HOP, 2 * 52], FP32, kind="ExternalOutput")

    with tile.TileContext(nc) as tc:
        with (
            tc.tile_pool(name="const", bufs=1) as cpool,
            tc.tile_pool(name="work", bufs=1) as wpool,
            tc.tile_pool(name="ps_x", bufs=1, space="PSUM") as ps_x,
            tc.tile_pool(name="ps_r", bufs=2, space="PSUM") as ps_r,
            tc.tile_pool(name="ps_u", bufs=2, space="PSUM") as ps_u,
            tc.tile_pool(name="ps_d", bufs=1, space="PSUM") as ps_d,
        ):
            frm = wpool.tile([F, 3 * T], FP32, tag="frm")
            nc.sync.dma_start(frm[:, :], xf[:, :])
            Fc = cpool.tile([F, 2 * F], FP32, tag="Fc")
            nc.sync.dma_start(Fc[:, :], fr_c[:, :])
            Ssb = cpool.tile([F, NJ * F], BF16, tag="Ssb")
            for q in range(NJ // PBK):
                nc.sync.dma_start(Ssb[:, q * PBK * F:(q + 1) * PBK * F],
                                  smat[:, q * PBK * F:(q + 1) * PBK * F])
            Msb = cpool.tile([F, NJ * 2 * F], BF16, tag="Msb")
            for c in range(NCH):
                nc.gpsimd.dma_start(Msb[:, c * CHJ * 2 * F:(c + 1) * CHJ * 2 * F],
                                    mst[:, c * CHJ * 2 * F:(c + 1) * CHJ * 2 * F])
            Gc = cpool.tile([F, 2 * F], BF16, tag="Gc")
            nc.gpsimd.dma_start(Gc[:, :], gr_c[:, :])
            sv = cpool.tile([HOP, 52], FP32, tag="sv")
            nc.gpsimd.dma_start(sv[:, :], svec[:, :])

            # ---- STFT (fp32) -> X bf16 [Xr(52) | Xi(52)] ----
            Xp = ps_x.tile([F, 2 * T], FP32, tag="Xp")
            nc.tensor.matmul(Xp[:, :], Fc[:, 0:F], frm[:, T:3 * T], start=True, stop=False)
            nc.tensor.matmul(Xp[:, :], Fc[:, F:2 * F], frm[:, 0:2 * T], start=False, stop=True)
            Xsb = wpool.tile([F, 2 * TP], BF16, tag="Xsb")
            Xsv = Xsb[:, :].rearrange("p (c t) -> p c t", c=2)
            nc.scalar.activation(Xsv[:, :, 0:T],
                                 Xp[:, :].rearrange("p (c t) -> p c t", c=2),
                                 mybir.ActivationFunctionType.Copy)
            Xrhs = bass.AP(tensor=Xsb[:, :].tensor, offset=Xsb[:, :].offset,
                           ap=[[2 * TP, F], [TP, 2], [1, T]])

            # plane-major per-chunk stacks: R/U = [r-block | i-block], blocks CHJ*TP
            # C/V = [negi-block | r-block | i-block]
            BL = CHJ * TP
            Rsb, Csb, Usb, Vsb = [], [], [], []
            for c in range(NCH):
                Rsb.append(wpool.tile([F, 2 * BL], BF16, tag=f"Rsb{c}", name=f"Rsb{c}"))
                Csb.append(wpool.tile([F, 3 * BL], BF16, tag=f"Csb{c}", name=f"Csb{c}"))
                Usb.append(wpool.tile([F, 2 * BL], BF16, tag=f"Usb{c}", name=f"Usb{c}"))
                Vsb.append(wpool.tile([F, 3 * BL], BF16, tag=f"Vsb{c}", name=f"Vsb{c}"))
            sA = wpool.tile([F, BL], BF16, tag="sA")
            sB = wpool.tile([F, BL], BF16, tag="sB")
            sC = wpool.tile([F, BL], BF16, tag="sC")
            sD = wpool.tile([F, BL], BF16, tag="sD")
            sPR = wpool.tile([F, BL], BF16, tag="sPR")
            sPI = wpool.tile([F, BL], BF16, tag="sPI")

            Dp = ps_d.tile([F, GJ * 2 * T], FP32, tag="Dp")  # 5 accumulated [dr|di] pairs

            TT = nc.vector.tensor_tensor
            TG = nc.gpsimd.tensor_tensor
            MUL = mybir.AluOpType.mult
            ADD = mybir.AluOpType.add
            SUB = mybir.AluOpType.subtract
            CPY = mybir.ActivationFunctionType.Copy

            for c in range(NCH):
                Rc, Cc, Uc, Vc = Rsb[c], Csb[c], Usb[c], Vsb[c]
                # ---- R: permutation matmuls, PBK per bank, plane-major evict ----
                for bk in range(CHJ // PBK):
                    Rp = ps_r.tile([F, PBK * 2 * T], FP32, tag="Rp")
                    for s in range(PBK):
                        j = c * CHJ + bk * PBK + s
                        nc.tensor.matmul(Rp[:, s * 2 * T:(s + 1) * 2 * T],
                                         Ssb[:, j * F:(j + 1) * F],
                                         Xrhs, start=True, stop=True)
                    # psum [s, c2, t] -> Rsb [c2-block, (bk*PBK+s)*TP + t]
                    dst = bass.AP(tensor=Rc[:, :].tensor,
                                  offset=Rc[:, :].offset + bk * PBK * TP,
                                  ap=[[2 * BL, F], [TP, PBK], [BL, 2], [1, T]])
                    nc.scalar.activation(
                        dst, Rp[:, :].rearrange("p (s c2 t) -> p s c2 t", s=PBK, c2=2),
                        CPY)

                Rrf = Rc[:, 0:BL]
                Rif = Rc[:, BL:2 * BL]
                vPR = sPR[:, :].rearrange("p (j t) -> p j t", j=CHJ)
                vPI = sPI[:, :].rearrange("p (j t) -> p j t", j=CHJ)

                # ---- C_pre = X * conj(R)  (flat 2D ops; Xt = tiled X copies) ----
                if c == 0:
                    Xtr = wpool.tile([F, BL], BF16, tag="Xtr")
                    Xti = wpool.tile([F, BL], BF16, tag="Xti")
                    nc.scalar.activation(
                        Xtr[:, :].rearrange("p (j t) -> p j t", j=CHJ),
                        Xsb[:, None, 0:TP].to_broadcast([F, CHJ, TP]), CPY)
                    nc.scalar.activation(
                        Xti[:, :].rearrange("p (j t) -> p j t", j=CHJ),
                        Xsb[:, None, TP:2 * TP].to_broadcast([F, CHJ, TP]), CPY)
                TT(sA[:, :], Xtr[:, :], Rrf, MUL)
                TT(sB[:, :], Xti[:, :], Rif, MUL)
                TT(sPR[:, :], sA[:, :], sB[:, :], ADD)
                TG(sC[:, :], Xti[:, :], Rrf, MUL)
                TG(sD[:, :], Xtr[:, :], Rif, MUL)
                TG(sPI[:, :], sC[:, :], sD[:, :], SUB)

                # ---- C = C_pre + roll_t;  blocks [CiN | Cr | Ci] ----
                CrB = Cc[:, BL:2 * BL].rearrange("p (j t) -> p j t", j=CHJ)
                CiB = Cc[:, 2 * BL:3 * BL].rearrange("p (j t) -> p j t", j=CHJ)
                TT(CrB[:, :, 1:T], vPR[:, :, 1:T], vPR[:, :, 0:T - 1], ADD)
                TT(CrB[:, :, 0:1], vPR[:, :, 0:1], vPR[:, :, T - 1:T], ADD)
                TG(CiB[:, :, 1:T], vPI[:, :, 1:T], vPI[:, :, 0:T - 1], ADD)
                TG(CiB[:, :, 0:1], vPI[:, :, 0:1], vPI[:, :, T - 1:T], ADD)
                nc.scalar.activation(Cc[:, 0:BL], Cc[:, 2 * BL:3 * BL], CPY, scale=-1.0)

                # ---- stage-1: U_j = Mr.T@[Cr|Ci] + Mi.T@[CiN|Cr] ----
                for bk in range(CHJ // PBK):
                    Up = ps_u.tile([F, PBK * 2 * T], FP32, tag="Up")
                    for s in range(PBK):
                        jj = bk * PBK + s
                        j = c * CHJ + jj
                        rhs1 = bass.AP(tensor=Cc[:, :].tensor,
                                       offset=Cc[:, :].offset + BL + jj * TP,
                                       ap=[[3 * BL, F], [BL, 2], [1, T]])
                        rhs2 = bass.AP(tensor=Cc[:, :].tensor,
                                       offset=Cc[:, :].offset + jj * TP,
                                       ap=[[3 * BL, F], [BL, 2], [1, T]])
                        nc.tensor.matmul(Up[:, s * 2 * T:(s + 1) * 2 * T],
                                         Msb[:, (2 * j) * F:(2 * j + 1) * F],
                                         rhs1, start=True, stop=False)
                        nc.tensor.matmul(Up[:, s * 2 * T:(s + 1) * 2 * T],
                                         Msb[:, (2 * j + 1) * F:(2 * j + 2) * F],
                                         rhs2, start=False, stop=True)
                    dst = bass.AP(tensor=Uc[:, :].tensor,
                                  offset=Uc[:, :].offset + bk * PBK * TP,
                                  ap=[[2 * BL, F], [TP, PBK], [BL, 2], [1, T]])
                    nc.scalar.activation(
                        dst, Up[:, :].rearrange("p (s c2 t) -> p s c2 t", s=PBK, c2=2),
                        CPY)

                # ---- stage-2: V = U * R;  blocks [ViN | Vr | Vi]  (flat 2D) ----
                Urf = Uc[:, 0:BL]
                Uif = Uc[:, BL:2 * BL]
                TT(sA[:, :], Urf, Rrf, MUL)
                TT(sB[:, :], Uif, Rif, MUL)
                TT(Vc[:, BL:2 * BL], sA[:, :], sB[:, :], SUB)
                TG(sC[:, :], Urf, Rif, MUL)
                TG(sD[:, :], Uif, Rrf, MUL)
                TG(Vc[:, 2 * BL:3 * BL], sC[:, :], sD[:, :], ADD)
                nc.scalar.activation(Vc[:, 0:BL], Vc[:, 2 * BL:3 * BL], CPY, scale=-1.0)

                # ---- merged G-matmuls: accumulate into 5 [dr|di] pairs ----
                for gpass in range(2):
                    for h in range(CHJ // GJ):
                        base = (BL if gpass == 0 else 0) + h * GJ * TP
                        rhs = bass.AP(tensor=Vc[:, :].tensor,
                                      offset=Vc[:, :].offset + base,
                                      ap=[[3 * BL, F], [TP, GJ], [BL, 2], [1, T]])
                        nc.tensor.matmul(
                            Dp[:, :].rearrange("p (s c2 t) -> p s c2 t", s=GJ, c2=2),
                            Gc[:, gpass * F:(gpass + 1) * F], rhs,
                            start=(c == 0 and gpass == 0 and h == 0),
                            stop=(c == NCH - 1 and gpass == 1 and h == CHJ // GJ - 1))

            # ---- reduce 5 pairs + overlap-add + scale (fp32) ----
            D5 = wpool.tile([F, GJ * 2 * T], FP32, tag="D5")
            nc.scalar.activation(D5[:, :], Dp[:, :], CPY)
            tE = wpool.tile([F, 4 * T], FP32, tag="tE")
            TT(tE[:, :], D5[:, 0:4 * T], D5[:, 4 * T:8 * T], ADD)       # p0+p2, p1+p3
            tF = wpool.tile([F, 2 * T], FP32, tag="tF")
            TT(tF[:, :], tE[:, 0:2 * T], tE[:, 2 * T:4 * T], ADD)
            Dsb = wpool.tile([F, 2 * T], FP32, tag="Dsb")
            TT(Dsb[:, :], tF[:, :], D5[:, 8 * T:10 * T], ADD)

            S2 = wpool.tile([HOP, 2 * T], FP32, tag="S2")
            nc.sync.dma_start(S2[:, :], Dsb[HOP:F, :])
            Y = wpool.tile([HOP, 2 * 52], FP32, tag="Y")
            S1v = Dsb[0:HOP, :].rearrange("p (c t) -> p c t", c=2)
            S2v = S2[:, :].rearrange("p (c t) -> p c t", c=2)
            Yv = Y[:, :].rearrange("p (c t) -> p c t", c=2)
            TT(Yv[:, :, 1:T], S1v[:, :, 1:T], S2v[:, :, 0:T - 1], ADD)
            nc.scalar.activation(Yv[:, :, 0:1], S1v[:, :, 0:1], CPY)
            nc.scalar.activation(Yv[:, :, T:52], S2v[:, :, T - 1:T], CPY)
            TT(Yv, Yv, sv[:, None, :].to_broadcast([HOP, 2, 52]), MUL)
            nc.sync.dma_start(yv[:, :], Y[:, :])
    return nc


# ---------------- host side ----------------

def _host_consts():
    W, G = _dft_consts()
    fr_c = np.concatenate([W.real, W.imag], axis=1).astype(np.float32)
    gr_c = np.concatenate([G.real, G.imag], axis=1).astype(bfloat16)
    cov = np.zeros(L)
    idx = (np.arange(T)[:, None] * HOP + np.arange(F)[None, :]).reshape(-1)
    np.add.at(cov, idx, 1.0)
    cov = np.where(cov > 0, cov, 1.0)
    return fr_c, gr_c, cov


def _smat_for(n2_list):
    S = np.zeros((NJ, F, F), np.float32)
    g = np.arange(F)
    for j, n2 in enumerate(n2_list):
        S[j, (g - n2) % F, g] = 1.0
    return np.ascontiguousarray(S.transpose(1, 0, 2).reshape(F, NJ * F)).astype(bfloat16)


def _mst_for(n2_list, w2):
    Ms = np.zeros((NJ, 2, F, F), np.float32)
    g = np.arange(F)[:, None]
    f = np.arange(F)[None, :]
    n1 = ((f - g + 20) % F) - 20
    valid = (n1 >= -20) & (n1 <= 19)
    n1c = np.clip(n1 + 20, 0, 39)
    for j, n2 in enumerate(n2_list):
        col = w2[:, n2 + 20]
        Ms[j, 0] = np.where(valid, col.real[n1c], 0.0)
        Ms[j, 1] = np.where(valid, col.imag[n1c], 0.0)
    return np.ascontiguousarray(
        Ms.transpose(2, 0, 1, 3).reshape(F, NJ * 2 * F)).astype(bfloat16)


def _frame(sig):
    idx = np.arange(T)[None, :] * HOP + np.arange(F)[:, None]   # [j, t]
    return sig[idx].astype(np.float32)


def make_in_maps(x_real, x_imag, task_info, w_real, w_imag):
    fr_c, gr_c, cov = _host_consts()
    b, _, m = x_real.shape
    P = np.power(10.0, task_info[:, 0] / 10.0) / m
    w2 = (np.asarray(w_real) + 1j * np.asarray(w_imag)).reshape(40, 40)
    smats = [_smat_for(nl) for nl in N2_LISTS]
    msts = [_mst_for(nl, w2) for nl in N2_LISTS]

    tp = np.arange(52)[None, :]
    tau = np.arange(HOP)[:, None]
    l = HOP * tp + tau
    svs = [(P[bb] / cov[l]).astype(np.float32) for bb in range(b)]

    in_maps, shards = [], []
    for bb in range(b):
        for mm in range(m):
            fr_ = _frame(x_real[bb, :, mm])
            fi_ = _frame(x_imag[bb, :, mm])
            xfv = np.concatenate([-fi_, fr_, fi_], axis=1).astype(np.float32)
            for h in range(2):
                in_maps.append({
                    "xf": xfv,
                    "fr_c": fr_c,
                    "gr_c": gr_c,
                    "smat": smats[h],
                    "mst": msts[h],
                    "svec": svs[bb],
                })
                shards.append((bb, mm, h))
    return in_maps, shards, P, cov


_NC_CACHE = {}


def kernel(x_real, x_imag, task_info, w_real, w_imag, b_real, b_imag):
    x_real = np.asarray(x_real)
    x_imag = np.asarray(x_imag)
    task_info = np.asarray(task_info)
    b, Lx, m = x_real.shape
    assert (b, Lx, m) == (2, L, 2)

    if "nc" not in _NC_CACHE:
        nc_ = build_program(debug=False)
        nc_.compile()
        _NC_CACHE["nc"] = nc_
    nc = _NC_CACHE["nc"]

    in_maps, shards, P, cov = make_in_maps(x_real, x_imag, task_info, w_real, w_imag)
    from concourse.bass_utils import run_bass_kernel_spmd
    res = run_bass_kernel_spmd(nc, in_maps, list(range(8))).results

    x = (x_real + 1j * x_imag).astype(np.complex64)
    out = x.copy()
    bias = complex(np.asarray(b_real)[0], np.asarray(b_imag)[0])
    bias_sig = np.zeros(L, np.complex64)
    bias_sig[np.arange(T) * HOP] = bias
    bias_sig /= cov
    for i, (bb, mm, h) in enumerate(shards):
        yvv = res[i]["yv"]          # [40, 104] = [tau, (yr(52) | yi(52))]
        yr = yvv[:, 0:52].T.ravel()[:L]
        yi = yvv[:, 52:104].T.ravel()[:L]
        out[bb, :, mm] += yr + 1j * yi
    for bb in range(b):
        for mm in range(m):
            out[bb, :, mm] += (P[bb] * bias_sig).astype(np.complex64)
    return out[:, 20:L - 20, :]
